# revision 1
# baseline (speedup 1.0000x reference)
"""AttentionNet (BiDAF-style) Trainium2 Bass kernel.

KEY STRUCTURE (faithful to the reference): every GRU scans over axis 0 of a
batch-first tensor — i.e. the recurrence runs over B=32 steps, while the
T=400 positions (and J=30 query positions) are independent lanes.

Sharding: the 400 context lanes are split 50/core across 8 cores; the 30
query lanes are replicated (cheap, and every core needs full Q for the
attention). Params replicated. Per-core compute is feature-major bf16 with
fp32 PSUM. Global reductions (softmax over all 400 positions, the G/M sums)
use AllReduce collectives over small fp32 buffers.
"""
import numpy as np
import ml_dtypes

import concourse.bass as bass
import concourse.mybir as mybir
import concourse.tile as tile
from concourse.bass_utils import run_bass_kernel_spmd

F32 = mybir.dt.float32
BF16 = mybir.dt.float16  # fp16: same PE speed as bf16, 10-bit mantissa
AF = mybir.ActivationFunctionType
ALU = mybir.AluOpType
AX = mybir.AxisListType
BF = np.float16

B_TOT, T, J, ANS = 32, 400, 30, 400
NB = 32              # scan steps (batch dim of the reference)
NCORES = 8
LN = T // NCORES     # 50 context lanes per core
JW = 32              # padded q-lane width (30 + 2 zeros)
W_CQ = LN + JW       # 82: combined ctx+q lane width in the ctx layer

CFG = {"ctx": dict(kin=2, kc=2), "mod": dict(kin=16, kc=2), "p2g": dict(kin=4, kc=4)}

_uid = [0]

def _split_excess_waits(nc, max_waits=1):
    for func in nc.m.functions:
        for block in func.blocks:
            new_insts = []
            for inst in block.instructions:
                si = inst.sync_info
                if si is not None and si.on_wait and len(si.on_wait) > max_waits:
                    waits = list(si.on_wait)
                    excess, keep = waits[:-max_waits], waits[-max_waits:]
                    for i in range(0, len(excess), max_waits):
                        chunk = excess[i:i + max_waits]
                        _uid[0] += 1
                        new_insts.append(mybir.InstNoOp(
                            name=f"waitsplit_nop_{_uid[0]}", ins=[], outs=[],
                            engine=inst.engine,
                            sync_info=mybir.SyncInfo(on_wait=list(chunk), on_update=[])))
                    inst.sync_info = mybir.SyncInfo(on_wait=list(keep),
                                                    on_update=list(si.on_update or []))
                new_insts.append(inst)
            block.instructions[:] = new_insts


def build_nc(taps=()):
    nc = bass.Bass()
    RG = [list(range(NCORES))]

    def din(name, shape, dt=BF16):
        return nc.dram_tensor(name, shape, dt, kind="ExternalInput")

    x_all = din("x_all", [128, 2, NB, W_CQ])
    wih_dram = {k: din(f"{k}_wih", [128, 2, CFG[k]["kin"] * 3 * CFG[k]["kc"] * 128])
                for k in CFG}
    whh_dram = {k: din(f"{k}_whh", [128, 2, CFG[k]["kc"] * 3 * CFG[k]["kc"] * 128])
                for k in CFG}
    gib_dram = {k: din(f"{k}_gib", [128, 2, 3 * CFG[k]["kc"]], F32) for k in CFG}
    bhn_dram = {k: din(f"{k}_bhn", [128, 2, CFG[k]["kc"]], F32) for k in CFG}
    w123 = din("w123", [128, 4, 3], F32)
    p1_wT = din("p1_wT", [128, 21, ANS])
    p2_wT = din("p2_wT", [128, 25, ANS])
    ident_in = din("ident_in", [128, 128])
    identf_in = din("identf_in", [128, 128], F32)

    out_p1 = nc.dram_tensor("out_p1", [NB, ANS], F32, kind="ExternalOutput")
    out_p2 = nc.dram_tensor("out_p2", [NB, ANS], F32, kind="ExternalOutput")

    ncop = [0]
    def spread_copy(out, in_, bias=None):
        ncop[0] += 1
        if bias is not None:
            if ncop[0] % 2 == 0:
                nc.scalar.activation(out, in_, AF.Identity, bias=bias)
            else:
                nc.vector.tensor_scalar(out, in_, bias, None, op0=ALU.add)
        else:
            if ncop[0] % 2 == 0:
                nc.scalar.copy(out, in_)
            else:
                nc.vector.tensor_copy(out, in_)

    with tile.TileContext(nc) as tc:
      with tc.tile_pool(name="const", bufs=1) as constp, \
           tc.tile_pool(name="acts", bufs=1) as acts, \
           tc.tile_pool(name="cell", bufs=3) as cellp, \
           tc.tile_pool(name="ccdram", bufs=1, space="DRAM") as ccd:

        ident = constp.tile([128, 128], BF16, tag="ident")
        nc.sync.dma_start(ident[:], ident_in[:])
        identf = constp.tile([128, 128], F32, tag="identf")
        nc.sync.dma_start(identf[:], identf_in[:])
        ones_row = constp.tile([1, ANS], BF16, tag="ones_row")
        nc.vector.memset(ones_row[:], 1.0)

        from contextlib import ExitStack
        _escq = ExitStack()
        pcq = _escq.enter_context(tc.tile_pool(name="pCQ", bufs=1))
        CQ = pcq.tile([128, 4, NB + 2, W_CQ], BF16, tag="CQ")
        M = acts.tile([128, 4, NB + 2, LN], BF16, tag="M")
        gsum = acts.tile([128, 16, NB], F32, tag="gsum")
        msum = acts.tile([128, 4, NB], F32, tag="msum")
        m2sum = acts.tile([128, 8, NB], F32, tag="m2sum")
        for tl in (CQ, M):
            nc.vector.memset(tl[:], 0.0)

        def tap(name, src):
            if name in taps:
                to = nc.dram_tensor(f"tap_{name}", list(src.shape), src.dtype,
                                    kind="ExternalOutput")
                nc.sync.dma_start(to[:], src[:])

        def Cv(ch, b):
            return CQ[:, ch, b, 0:LN]
        def Qv(ch, b):
            return CQ[:, ch, b, LN:LN + J]

        # -------- gi precompute --------
        def gi_phase(name, x_mov, width, gi_tile, pool, psp, dirs=(0, 1)):
            kin, kc = CFG[name]["kin"], CFG[name]["kc"]
            gc = 3 * kc
            gib_sb = pool.tile([128, 2, gc], F32, tag=f"{name}_gib")
            nc.sync.dma_start(gib_sb[:], gib_dram[name][:])
            bchunk = max(1, 512 // width)
            for d in dirs:
                wih_sb = pool.tile([128, kin * gc * 128], BF16, tag=f"{name}_wih")
                nc.sync.dma_start(wih_sb[:], wih_dram[name][:, d, :])
                wv = wih_sb[:].rearrange("p (a g n) -> p a g n", a=kin, n=128)
                for b0 in range(0, NB, bchunk):
                    nb = min(bchunk, NB - b0)
                    for g in range(gc):
                        pt = psp.tile([128, 512], F32, tag="gi")
                        for k in range(kin):
                            nc.tensor.matmul(pt[:, :nb * width], wv[:, k, g, :],
                                             x_mov(k)[:, b0:b0 + nb, :],
                                             start=(k == 0), stop=(k == kin - 1))
                        spread_copy(
                            gi_tile[:, d if len(dirs) > 1 else 0, g, b0:b0 + nb, :],
                            pt[:, :nb * width].rearrange("p (b w) -> p b w", w=width),
                            bias=gib_sb[:, d, g:g + 1])

        # -------- recurrence --------
        def recur(name, whh_v, bhn_sb, gi_tile, out_tile, width, psp, dirs=(0, 1)):
            kc = CFG[name]["kc"]
            gc = 3 * kc

            def step_dir(d, b):
                gd = d if len(dirs) > 1 else 0
                koff = 0 if d == 0 else kc
                rd, wr = (b, b + 1) if d == 0 else (b + 2, b + 1)
                h_prev = out_tile[:, koff:koff + kc, rd, :]
                pgrz = psp.tile([128, 2 * kc * width], F32, tag=f"{name}rzp{d}")
                pgn = psp.tile([128, kc * width], F32, tag=f"{name}nps{d}")
                nc.tensor.matmul(pgrz[:], ident[:], gi_tile[:, gd, 0:2 * kc, b, :],
                                 start=True, stop=False, skip_group_check=True)
                for g in range(gc):
                    for k in range(kc):
                        if g < 2 * kc:
                            dst = pgrz[:, g * width:(g + 1) * width]
                            st = False
                        else:
                            dst = pgn[:, (g - 2 * kc) * width:(g - 2 * kc + 1) * width]
                            st = (k == 0)
                        nc.tensor.matmul(dst, whh_v[:, d, k, g, :], h_prev[:, k, :],
                                         start=st,
                                         stop=(g == gc - 1 and k == kc - 1),
                                         skip_group_check=True)
                rz = cellp.tile([128, 2 * kc, width], BF16, tag=f"{name}rz{d}")
                nc.scalar.activation(
                    rz[:], pgrz[:].rearrange("p (g w) -> p g w", w=width), AF.Sigmoid)
                tt = cellp.tile([128, kc, width], BF16, tag=f"{name}t{d}")
                for k in range(kc):
                    nc.vector.scalar_tensor_tensor(
                        tt[:, k, :], pgn[:, k * width:(k + 1) * width],
                        bhn_sb[:, d, k:k + 1], rz[:, k, :], op0=ALU.add, op1=ALU.mult)
                npre = cellp.tile([128, kc, width], BF16, tag=f"{name}npre{d}")
                nc.vector.tensor_tensor(npre[:], tt[:], gi_tile[:, gd, 2 * kc:, b, :],
                                        op=ALU.add)
                nt = cellp.tile([128, kc, width], BF16, tag=f"{name}n{d}")
                nc.scalar.activation(nt[:], npre[:], AF.Tanh)
                dd = cellp.tile([128, kc, width], BF16, tag=f"{name}d{d}")
                nc.vector.tensor_tensor(dd[:], h_prev, nt[:], op=ALU.subtract)
                ee = cellp.tile([128, kc, width], BF16, tag=f"{name}e{d}")
                nc.gpsimd.tensor_tensor(ee[:], rz[:, kc:2 * kc, :], dd[:], op=ALU.mult)
                nc.vector.tensor_tensor(out_tile[:, koff:koff + kc, wr, :],
                                        nt[:], ee[:], op=ALU.add)

            for s in range(NB):
                for d in dirs:
                    step_dir(d, s if d == 0 else NB - 1 - s)

        def load_whh(name, pool):
            kc = CFG[name]["kc"]
            gc = 3 * kc
            whh_sb = pool.tile([128, 2, kc * gc * 128], BF16, tag=f"{name}_whh")
            nc.sync.dma_start(whh_sb[:], whh_dram[name][:])
            bhn_sb = pool.tile([128, 2, kc], F32, tag=f"{name}_bhn")
            nc.sync.dma_start(bhn_sb[:], bhn_dram[name][:])
            return whh_sb[:].rearrange("p d (a g n) -> p d a g n", a=kc, n=128), bhn_sb

        def allreduce(sb_aps, op):
            tot = sum(int(np.prod(a.shape)) for a in sb_aps)
            _uid[0] += 1
            cin = ccd.tile([tot], F32, tag=f"cc_in{_uid[0]}", bufs=1)
            cout = ccd.tile([tot], F32, tag=f"cc_out{_uid[0]}", bufs=1)
            off = 0
            for a in sb_aps:
                n = int(np.prod(a.shape))
                nc.sync.dma_start(
                    cin[off:off + n].rearrange("(p f) -> p f", p=a.shape[0]), a)
                off += n
            nc.gpsimd.collective_compute("AllReduce", op, replica_groups=RG,
                                         ins=[cin.opt()], outs=[cout.opt()])
            off = 0
            for a in sb_aps:
                n = int(np.prod(a.shape))
                nc.sync.dma_start(
                    a, cout[off:off + n].rearrange("(p f) -> p f", p=a.shape[0]))
                off += n

        # ================= ctx layer =================
        with tc.tile_pool(name="pctx", bufs=1) as pctx:
            xs = pctx.tile([128, 2, NB, W_CQ], BF16, tag="xs")
            nc.sync.dma_start(xs[:], x_all[:])
            gi_ctx = pctx.tile([128, 2, 6, NB, W_CQ], BF16, tag="gi_ctx")
            with tc.tile_pool(name="psgi_ctx", bufs=4, space="PSUM") as psp:
                gi_phase("ctx", lambda k: xs[:, k, :, :], W_CQ, gi_ctx, pctx, psp)
            whh_v, bhn_sb = load_whh("ctx", pctx)
            with tc.tile_pool(name="psrec_ctx", bufs=2, space="PSUM") as psp:
                recur("ctx", whh_v, bhn_sb, gi_ctx, CQ, W_CQ, psp)
        tap("CQ", CQ)

        # ================= attention =================
        with tc.tile_pool(name="pattn", bufs=1) as pa:
            from contextlib import ExitStack
            _esat = ExitStack()
            patmp = _esat.enter_context(tc.tile_pool(name="patmp", bufs=1))
            wv = constp.tile([128, 4, 3], F32, tag="wv")
            nc.sync.dma_start(wv[:], w123[:])
            w1b = constp.tile([128, 4, 1], BF16, tag="w1b")
            nc.vector.tensor_copy(w1b[:], wv[:, :, 0:1])

            cmul = patmp.tile([128, 4, NB, LN], BF16, tag="cmul")
            for ch in range(4):
                nc.vector.tensor_scalar(cmul[:, ch, :, :], CQ[:, ch, 1:NB + 1, 0:LN],
                                        wv[:, ch, 2:3], None, op0=ALU.mult)
            q3 = patmp.tile([128, 4, NB, JW], BF16, tag="q3")
            nc.vector.memset(q3[:], 0.0)
            for ch in range(4):
                nc.vector.tensor_scalar(q3[:, ch, :, 0:J],
                                        CQ[:, ch, 1:NB + 1, LN:LN + J],
                                        wv[:, ch, 2:3], None, op0=ALU.mult)

            gxq = pa.tile([128, 4, NB, LN], BF16, tag="gxq")

            with tc.tile_pool(name="psattn", bufs=2, space="PSUM") as psa:
                cw1 = patmp.tile([1, NB, LN], BF16, tag="cw1")
                for b0 in range(0, NB, 10):
                    nb = min(10, NB - b0)
                    pc = psa.tile([1, 512], F32, tag="psA")
                    for k in range(4):
                        nc.tensor.matmul(pc[:, :nb * LN], w1b[:, k, :],
                                         CQ[:, k, b0 + 1:b0 + 1 + nb, 0:LN],
                                         start=(k == 0), stop=(k == 3))
                    nc.vector.tensor_copy(
                        cw1[:, b0:b0 + nb, :],
                        pc[:, :nb * LN].rearrange("p (b w) -> p b w", w=LN))
                w2b = constp.tile([128, 4, 1], BF16, tag="w2b")
                nc.vector.tensor_copy(w2b[:], wv[:, :, 1:2])
                qw2p = patmp.tile([1, NB, JW], BF16, tag="qw2p")
                nc.vector.memset(qw2p[:], 0.0)
                for b0 in range(0, NB, 16):
                    pq = psa.tile([1, 512], F32, tag="psA")
                    for k in range(4):
                        nc.tensor.matmul(pq[:, :16 * J], w2b[:, k, :],
                                         CQ[:, k, b0 + 1:b0 + 17, LN:LN + J],
                                         start=(k == 0), stop=(k == 3))
                    nc.vector.tensor_copy(
                        qw2p[:, b0:b0 + 16, 0:J],
                        pq[:, :16 * J].rearrange("p (b w) -> p b w", w=J))

                s_sbT = patmp.tile([32, NB, LN], BF16, tag="s_sbT")
                smax_T = patmp.tile([64, NB], F32, tag="smax_T")
                for b in range(NB):
                    psT = psa.tile([32, 512], F32, tag="psA")
                    for k in range(4):
                        nc.tensor.matmul(psT[:, :LN], q3[:, k, b, :], Cv(k, b + 1),
                                         start=(k == 0), stop=False,
                                         skip_group_check=True)
                    nc.tensor.matmul(psT[0:J, :LN], ones_row[:, 0:J], cw1[:, b, :],
                                     start=False, stop=False, skip_group_check=True)
                    nc.tensor.matmul(psT[:, :LN], qw2p[:, b, :], ones_row[:, 0:LN],
                                     start=False, stop=True, skip_group_check=True)
                    spread_copy(s_sbT[:, b, :], psT[:, :LN])
                    pS = psa.tile([64, 32], F32, tag="psS")
                    for k in range(4):
                        nc.tensor.matmul(pS[0:LN, 0:J], cmul[:, k, b, :], Qv(k, b + 1),
                                         start=(k == 0), stop=False,
                                         skip_group_check=True)
                    nc.tensor.matmul(pS[0:LN, 0:J], cw1[:, b, :], ones_row[:, 0:J],
                                     start=False, stop=False, skip_group_check=True)
                    nc.tensor.matmul(pS[0:LN, 0:J], ones_row[:, 0:LN], qw2p[:, b, 0:J],
                                     start=False, stop=True, skip_group_check=True)
                    nc.vector.tensor_reduce(smax_T[0:LN, b:b + 1], pS[0:LN, 0:J],
                                            axis=AX.X, op=ALU.max)

                tap("s_sbT", s_sbT)
                tap("smax_T", smax_T)
                psm = psa.tile([NB, 64], F32, tag="psS")
                nc.tensor.transpose(psm[:, 0:64], smax_T[:, :], identf[0:64, 0:64])
                smax_bm = patmp.tile([NB, LN], F32, tag="smax_bm")
                nc.vector.tensor_copy(smax_bm[:], psm[0:NB, 0:LN])
                m_g = patmp.tile([NB, 1], F32, tag="m_g")
                nc.vector.tensor_reduce(m_g[:], smax_bm[:], axis=AX.X, op=ALU.max)
                allreduce([m_g[:]], ALU.max)
                nm_g = patmp.tile([NB, 1], F32, tag="nm_g")
                nc.vector.tensor_scalar_mul(nm_g[:], m_g[:], -1.0)
                e_bm = patmp.tile([NB, LN], BF16, tag="e_bm")
                lsum = patmp.tile([NB, 1], F32, tag="lsum")
                nc.scalar.activation(e_bm[:], smax_bm[:], AF.Exp, bias=nm_g[:],
                                     accum_out=lsum[:])
                e_d = ccd.tile([NB * LN], BF16, tag="e_d", bufs=1)
                nc.sync.dma_start(e_d[:].rearrange("(p f) -> p f", p=NB), e_bm[:])
                e_bc = patmp.tile([128, NB, LN], BF16, tag="e_bc")
                nc.sync.dma_start(
                    e_bc[:].rearrange("p b w -> p (b w)"),
                    bass.AP(tensor=e_d.tensor, offset=e_d.offset,
                            ap=[[0, 128], [1, NB * LN]]))
                tap("e_bm", e_bm)
                q2c = patmp.tile([128, 4, NB], F32, tag="q2c")
                scr = patmp.tile([128, 64], BF16, tag="scr")
                for ch in range(4):
                    for b in range(NB):
                        nc.vector.scalar_tensor_tensor(
                            scr[:, 0:LN], Cv(ch, b + 1), 1.0, e_bc[:, b, :],
                            op0=ALU.mult, op1=ALU.mult,
                            accum_out=q2c[:, ch, b:b + 1])

                c2q = pa.tile([128, 4, NB, LN], BF16, tag="c2q")
                for ch in range(4):
                    qbm = patmp.tile([32, NB, 128], BF16, tag="qbm")
                    for b in range(NB):
                        ptq = psa.tile([32, 128], BF16, tag="ptq")
                        nc.tensor.transpose(ptq[:],
                                            CQ[:, ch, b + 1, LN:LN + JW], ident[:])
                        spread_copy(qbm[:, b, :], ptq[:])
                    for b in range(NB):
                        pc2 = psa.tile([128, 64], F32, tag="pc2")
                        nc.tensor.matmul(pc2[:, 0:LN], qbm[:, b, :],
                                         s_sbT[:, b, :], start=True, stop=True)
                        spread_copy(c2q[:, ch, b, :], pc2[:, 0:LN])

            gxc = pa.tile([128, 4, NB, LN], BF16, tag="gxc")
            for ch in range(4):
                nc.vector.tensor_tensor(gxc[:, ch, :, :], CQ[:, ch, 1:NB + 1, 0:LN],
                                        c2q[:, ch, :, :], op=ALU.mult)

            def gpart(k):
                if k < 4:
                    return CQ[:, k, 1:NB + 1, 0:LN]
                if k < 8:
                    return c2q[:, k - 4, :, :]
                if k < 12:
                    return gxc[:, k - 8, :, :]
                return gxq[:, k - 12, :, :]

            for k in range(12):
                nc.vector.tensor_reduce(gsum[:, k, :], gpart(k), axis=AX.X, op=ALU.add)

            allreduce([lsum[:], q2c[:].rearrange("p a b -> p (a b)"),
                       gsum[:, 0:12, :].rearrange("p a b -> p (a b)")], ALU.add)

            rs = patmp.tile([NB, 1], F32, tag="rs")
            nc.vector.reciprocal(rs[:], lsum[:])
            rs_d = ccd.tile([NB], F32, tag="rs_d", bufs=1)
            nc.sync.dma_start(rs_d[:].rearrange("(p f) -> p f", p=NB), rs[:])
            rs_bc = patmp.tile([128, NB], F32, tag="rs_bc")
            nc.sync.dma_start(rs_bc[:],
                              bass.AP(tensor=rs_d.tensor, offset=rs_d.offset,
                                      ap=[[0, 128], [1, NB]]))
            q2cn = patmp.tile([128, 4, NB], F32, tag="q2cn")
            nc.vector.tensor_tensor(
                q2cn[:], q2c[:],
                bass.AP(tensor=rs_bc.tensor, offset=rs_bc.offset,
                        ap=[rs_bc.ap[0], [0, 4], rs_bc.ap[1]]),
                op=ALU.mult)
            nc.vector.tensor_tensor(gsum[:, 12:16, :], gsum[:, 0:4, :], q2cn[:],
                                    op=ALU.mult)
            for ch in range(4):
                for b in range(NB):
                    eng = nc.vector if (ch * NB + b) % 2 == 0 else nc.gpsimd
                    eng.tensor_scalar(gxq[:, ch, b, :], Cv(ch, b + 1),
                                      q2cn[:, ch, b:b + 1], None, op0=ALU.mult)
            tap("c2q", c2q)
            _esat.close()

            # ================= mod layer =================
            with tc.tile_pool(name="pmod", bufs=1) as pm:
                gi_mod = pm.tile([128, 2, 6, NB, LN], BF16, tag="gi_mod")
                with tc.tile_pool(name="psgi_mod", bufs=4, space="PSUM") as psp:
                    gi_phase("mod", gpart, LN, gi_mod, pm, psp)
                whh_v, bhn_sb = load_whh("mod", pm)
                with tc.tile_pool(name="psrec_mod", bufs=2, space="PSUM") as psp:
                    recur("mod", whh_v, bhn_sb, gi_mod, M, LN, psp)
        _escq.close()
        tap("gsum", gsum)
        tap("M", M)

        for k in range(4):
            nc.vector.tensor_reduce(msum[:, k, :], M[:, k, 1:NB + 1, :],
                                    axis=AX.X, op=ALU.add)
        allreduce([msum[:].rearrange("p a b -> p (a b)")], ALU.add)

        # ================= heads + p2g =================
        with tc.tile_pool(name="phead", bufs=1) as ph:
            def head(w_dram, nchunk, srcs, out_dram, pstag):
                w_sb = ph.tile([128, nchunk, ANS], BF16, tag="w_head", bufs=1)
                nc.sync.dma_start(w_sb[:], w_dram[:])
                gm = ph.tile([128, nchunk, NB], BF16, tag=f"gm_{pstag}")
                nc.vector.memset(gm[:, nchunk - 1, :], 0.0)
                nc.vector.memset(gm[0:1, nchunk - 1, :], 1.0)
                off = 0
                for s in srcs:
                    nchk = s.shape[1]
                    nc.vector.tensor_copy(gm[:, off:off + nchk, :], s[:])
                    off += nchk
                with tc.tile_pool(name=f"psh_{pstag}", bufs=1, space="PSUM") as psh:
                    ps_ = psh.tile([NB, ANS], F32, tag=f"ps{pstag}")
                    for k in range(nchunk):
                        nc.tensor.matmul(ps_[:], gm[:, k, :], w_sb[:, k, :],
                                         start=(k == 0), stop=(k == nchunk - 1))
                    mx = ph.tile([NB, 1], F32, tag=f"mx{pstag}")
                    nc.vector.tensor_reduce(mx[:], ps_[:], axis=AX.X, op=ALU.max)
                    nmx = ph.tile([NB, 1], F32, tag=f"nmx{pstag}")
                    nc.vector.tensor_scalar_mul(nmx[:], mx[:], -1.0)
                    sm = ph.tile([NB, 1], F32, tag=f"sm{pstag}")
                    ee = ph.tile([NB, ANS], F32, tag=f"e{pstag}")
                    nc.scalar.activation(ee[:], ps_[:], AF.Exp, bias=nmx[:],
                                         accum_out=sm[:])
                    rr = ph.tile([NB, 1], F32, tag=f"r{pstag}")
                    nc.vector.reciprocal(rr[:], sm[:])
                    po = ph.tile([NB, ANS], F32, tag=f"po{pstag}")
                    nc.vector.tensor_scalar(po[:], ee[:], rr[:], None, op0=ALU.mult)
                    nc.sync.dma_start(out_dram[:], po[:])

            head(p1_wT, 21, [gsum, msum], out_p1, "1")

            M2 = ph.tile([128, 8, NB + 2, LN], BF16, tag="M2")
            nc.vector.memset(M2[:], 0.0)
            with tc.tile_pool(name="pp2g", bufs=1) as pp:
                whh_v, bhn_sb = load_whh("p2g", pp)
                for d in (0, 1):
                    gi_p2g = pp.tile([128, 1, 12, NB, LN], BF16, tag="gi_p2g")
                    with tc.tile_pool(name="psgi_p2g", bufs=4, space="PSUM") as psp:
                        gi_phase("p2g", lambda k: M[:, k, 1:NB + 1, :], LN, gi_p2g,
                                 pp, psp, dirs=(d,))
                    with tc.tile_pool(name="psrec_p2g", bufs=2, space="PSUM") as psp:
                        recur("p2g", whh_v, bhn_sb, gi_p2g, M2, LN, psp, dirs=(d,))
            tap("M2", M2)

            for k in range(8):
                nc.vector.tensor_reduce(m2sum[:, k, :], M2[:, k, 1:NB + 1, :],
                                        axis=AX.X, op=ALU.add)
            allreduce([m2sum[:].rearrange("p a b -> p (a b)")], ALU.add)

            head(p2_wT, 25, [gsum, m2sum], out_p2, "2")

    _split_excess_waits(nc)
    return nc


# ---------------------------------------------------------------- host prep
def _fm_stat(wT, kin, gc):
    din, dout = wT.shape
    assert din == kin * 128 and dout == gc * 128, (wT.shape, kin, gc)
    return np.ascontiguousarray(
        wT.reshape(kin, 128, gc, 128).transpose(1, 0, 2, 3).reshape(128, -1)
    ).astype(BF)


def _prep_params(i):
    out = {}
    for name in CFG:
        kin, kc = CFG[name]["kin"], CFG[name]["kc"]
        gc = 3 * kc
        wih = np.asarray(i[f"{name}_Wih"], np.float32)
        whh = np.asarray(i[f"{name}_Whh"], np.float32)
        bih = np.asarray(i[f"{name}_bih"], np.float32)
        bhh = np.asarray(i[f"{name}_bhh"], np.float32)
        out[f"{name}_wih"] = np.stack(
            [_fm_stat(wih[d].T, kin, gc) for d in range(2)], axis=1)
        out[f"{name}_whh"] = np.stack(
            [_fm_stat(whh[d].T, kc, gc) for d in range(2)], axis=1)
        H = kc * 128
        gib = np.zeros((128, 2, gc), np.float32)
        bhn = np.zeros((128, 2, kc), np.float32)
        for d in range(2):
            v = bih[d].copy()
            v[:2 * H] += bhh[d][:2 * H]
            gib[:, d, :] = v.reshape(gc, 128).T
            bhn[:, d, :] = bhh[d][2 * H:].reshape(kc, 128).T
        out[f"{name}_gib"] = gib
        out[f"{name}_bhn"] = bhn

    W = np.asarray(i["W"], np.float32)
    out["w123"] = np.ascontiguousarray(np.stack(
        [W[0:512].reshape(4, 128).T, W[512:1024].reshape(4, 128).T,
         W[1024:1536].reshape(4, 128).T], axis=-1)).astype(np.float32)

    def headw(w, b, nchunk):
        wT = np.asarray(w, np.float32).T
        K = wT.shape[0]
        arr = np.zeros((128, nchunk, ANS), np.float32)
        arr[:, :K // 128, :] = wT.reshape(K // 128, 128, ANS).transpose(1, 0, 2)
        arr[0, nchunk - 1, :] = np.asarray(b, np.float32)
        return arr.astype(BF)

    out["p1_wT"] = headw(i["p1_w"], i["p1_b"], 21)
    out["p2_wT"] = headw(i["p2_w"], i["p2_b"], 25)
    out["ident_in"] = np.eye(128, dtype=np.float32).astype(BF)
    out["identf_in"] = np.eye(128, dtype=np.float32)
    return out


def _prep_x(embd_ctx, embd_q):
    xc = np.asarray(embd_ctx, np.float32)
    xq = np.asarray(embd_q, np.float32)
    per_core = []
    for c in range(NCORES):
        x = np.zeros((NB, W_CQ, 256), np.float32)
        x[:, 0:LN, :] = xc[:, c * LN:(c + 1) * LN, :]
        x[:, LN:LN + J, :] = xq
        xf = x.transpose(2, 0, 1)
        per_core.append(np.ascontiguousarray(
            xf.reshape(2, 128, NB, W_CQ).transpose(1, 0, 2, 3)).astype(BF))
    return per_core


_BUILD_CACHE = {}

def _get_nc(taps=()):
    key = tuple(taps)
    if key not in _BUILD_CACHE:
        _BUILD_CACHE[key] = build_nc(key)
    return _BUILD_CACHE[key]


def make_in_maps(inputs):
    params = _prep_params(inputs)
    xs = _prep_x(inputs["embd_ctx"], inputs["embd_q"])
    in_maps = []
    for c in range(NCORES):
        m = dict(params)
        m["x_all"] = xs[c]
        in_maps.append(m)
    return in_maps


def kernel(**inputs):
    nc = _get_nc()
    in_maps = make_in_maps(inputs)
    res = run_bass_kernel_spmd(nc, in_maps, core_ids=list(range(NCORES))).results
    p1 = np.asarray(res[0]["out_p1"], np.float32)
    p2 = np.asarray(res[0]["out_p2"], np.float32)
    return p1, p2

